# revision 25
# baseline (speedup 1.0000x reference)
"""Causal self-attention (B=4, T=2048, C=1024, H=16) on 8 NeuronCores.

Sharding: data-parallel over batch (4) x tensor-parallel over heads (2 groups
of 8 heads) = 8 cores. Each core computes QKV for its 8 heads, causal
attention, and a partial output projection (row-parallel). Host sums the two
partial projections per batch and adds b_proj.

All matmul operands fp16 (PE multiplies at ~fp22 internally; fp16's 11-bit
mantissa keeps rel-err ~5e-4, well under the 2e-2 gate), fp32 PSUM accum.

v2 (HW-measured cost model: each self-loading matmul pays ~150ns serial
LDWEIGHTS+dispatch on top of N/2.4GHz streaming; two K=64 matmuls on
alternating row-halves run CONCURRENTLY at 2 cols/cycle, ~382ns/pair@N=512):
  - x is pre-transposed on the HOST (no device DMA-transpose; kills ~65us of
    DMA-queue occupancy), loaded per 512-row chunk as one contiguous DMA.
  - scores for head pairs (2h, 2h+1) are emitted as adjacent K=64 matmuls on
    row-halves 0-63/64-127 of the same kT/qT tile -> row-group concurrency
    halves score time. Both heads share one 1024-col PSUM tile + one exp.
  - v bias via tensor_tensor add on DVE during the PSUM->SBUF copy (drops the
    16 K=1 bias matmuls); causal masking via 2D-pattern affine_selects that
    cover both heads of a pair in one Pool op.
  - weight/x DMAs split into slabs and spread over the SP/ACT/DVE/Pool queues
    so the first matmul starts ~1us in; out DMAs round-robin over queues.
  - phase-1 chunk ct+1 and projection of chunk c-1 are dripped into the
    attention stream of chunk c so PE never starves while ACT does exp
    (ACT total ~147us << PE total, so PE stays the critical path).
"""

from contextlib import nullcontext

import numpy as np

import concourse.bass as bass
import concourse.mybir as mybir
from concourse import bacc
from concourse.tile import TileContext
from concourse.bass_utils import run_bass_kernel_spmd

B, T, C, H, D = 4, 2048, 1024, 16, 64
CQ = 512          # q (or k or v) channels per core = 8 heads * 64
HPC = 8           # heads per core
F32 = mybir.dt.float32
F16 = mybir.dt.float16
Exp = mybir.ActivationFunctionType.Exp
is_ge = mybir.AluOpType.is_ge

TCH = 512         # phase-1 T-chunk
NCH = T // TCH    # 4 chunks
VSTR = HPC * (D + 1)   # 520: v_ext per-T-block stride (8 heads x 65)
PAIRED = True
GMODE = "pair"


def build_nc(loop_n=1):
    """loop_n > 1 wraps the whole kernel in a device-side repeat loop
    (benchmarking only -- output is identical every iteration)."""
    nc = bacc.Bacc("TRN2", target_bir_lowering=False, debug=False, num_devices=8)

    # host-pretransposed x, chunk-major: xT[p, ct*4096 + kk*512 + t]
    xT = nc.dram_tensor("xT", [128, NCH * 8 * TCH], F16, kind="ExternalInput")
    # w_qk packed [p, kk*1024 + n] = w_qk[kk*128+p, n]
    w_qk = nc.dram_tensor("w_qk", [128, 8 * 1024], F16, kind="ExternalInput")
    w_v = nc.dram_tensor("w_v", [128, 8 * 512], F16, kind="ExternalInput")
    b_qk = nc.dram_tensor("b_qk", [128, 8], F32, kind="ExternalInput")
    b_vb = nc.dram_tensor("b_vb", [128, CQ], F16, kind="ExternalInput")
    w_pj = nc.dram_tensor("w_pj", [128, 4 * 1024], F16, kind="ExternalInput")
    out = nc.dram_tensor("out", [T, C], F16, kind="ExternalOutput")

    with TileContext(nc) as tc:
        with (
            tc.tile_pool(name="const", bufs=1) as pc,
            tc.tile_pool(name="persist", bufs=1) as pp,
            tc.tile_pool(name="work", bufs=2) as pw,
            tc.tile_pool(name="psum", bufs=2, space="PSUM") as ps,
            tc.For_i(0, loop_n, 1) if loop_n > 1 else nullcontext(),
        ):
            # ---- persistent activations ----
            qT = [pp.tile([128, T], F16, name=f"qT{m}") for m in range(4)]
            kT = [pp.tile([128, T], F16, name=f"kT{m}") for m in range(4)]
            yT = [pp.tile([128, T], F16, name=f"yT{m}") for m in range(4)]
            v_ext = pp.tile([128, (T // 128) * VSTR], F16, name="v_ext")

            w_qk_sb = pc.tile([128, 8 * 1024], F16, name="w_qk_sb")
            b_qk2 = pc.tile([128, 8], F32, name="b_qk2")
            b_vb_sb = pc.tile([128, CQ], F16, name="b_vb_sb")
            w_v_sb = pc.tile([128, 8 * 512], F16, name="w_v_sb")
            w_pj_sb = pc.tile([128, 4 * 1024], F16, name="w_pj_sb")

            # DMA queue round-robin (keeps any single DGE queue from
            # serializing the kernel)
            qs = [nc.sync, nc.scalar]

            # ---- startup DMAs: first matmul needs w_qk slab 0 + x chunk 0
            for kk in range(8):
                qs[kk % 2].dma_start(
                    out=w_qk_sb[:, kk * 1024:(kk + 1) * 1024],
                    in_=w_qk[:, kk * 1024:(kk + 1) * 1024],
                )
            nc.sync.dma_start(
                out=w_v_sb[:], in_=w_v[:])
            nc.scalar.dma_start(out=b_qk2[:], in_=b_qk[:])
            nc.scalar.dma_start(out=b_vb_sb[:], in_=b_vb[:])

            # ones column of v_ext makes PV emit softmax denominators
            v_ones = v_ext[:].rearrange(
                "p (t i d) -> p t i d", i=HPC, d=D + 1
            )[:, :, :, D:D + 1]
            nc.gpsimd.memset(v_ones, 1.0)

            def phase1_chunk_steps(ct, xq):
                """Yield emission closures for one phase-1 chunk."""
                T0 = ct * TCH
                xT_c = pw.tile([128, 8 * TCH], F16, name="xT_c", tag="xT_c", bufs=2)

                def xload():
                    xq.dma_start(
                        out=xT_c[:],
                        in_=xT[:, ct * 8 * TCH:(ct + 1) * 8 * TCH],
                    )

                def qk(m0):
                    # q,k: out^T layout [ch, T-chunk]; bias fused into copy
                    for m in range(m0, m0 + 2):
                        qk_ps = ps.tile([128, TCH], F32, name="qk_ps", tag="mm", bufs=3)
                        for kk in range(8):
                            nc.tensor.matmul(
                                qk_ps[:],
                                w_qk_sb[:, kk * 1024 + m * 128:kk * 1024 + (m + 1) * 128],
                                xT_c[:, kk * TCH:(kk + 1) * TCH],
                                start=(kk == 0),
                                stop=(kk == 7),
                            )
                        dst = qT[m] if m < 4 else kT[m - 4]
                        nc.vector.tensor_scalar_add(
                            dst[:, T0:T0 + TCH], qk_ps[:], b_qk2[:, m:m + 1]
                        )

                def vpart(tt):
                    # v: natural layout [T-block, ch], interleaved into v_ext
                    v_ps = ps.tile([128, CQ], F32, name="v_ps", tag="mm", bufs=3)
                    for kk in range(8):
                        nc.tensor.matmul(
                            v_ps[:],
                            xT_c[:, kk * TCH + tt * 128:kk * TCH + (tt + 1) * 128],
                            w_v_sb[:, kk * 512:(kk + 1) * 512],
                            start=(kk == 0),
                            stop=(kk == 7),
                        )
                    tb = ct * (TCH // 128) + tt
                    dst = v_ext[:, tb * VSTR:(tb + 1) * VSTR].rearrange(
                        "p (i d) -> p i d", d=D + 1
                    )[:, :, 0:D]
                    src = v_ps[:].rearrange("p (i d) -> p i d", d=D)
                    nc.vector.tensor_copy(dst, src)

                yield xload
                yield lambda: qk(0)
                yield lambda: qk(2)
                yield lambda: qk(4)
                yield lambda: qk(6)
                for t0 in range(0, TCH // 128):
                    yield lambda t0=t0: vpart(t0)

            def attention_v1(c, i):
                """v1-style single-head attention (bisection reference)."""
                m = i // 2
                p0 = 64 * (i % 2)
                y_ps = ps.tile([D + 1, 512], F32, name="y_ps", tag="psy", bufs=2)
                first_pv = [True]

                def vslice(tkb):
                    return v_ext[
                        :, tkb * VSTR + i * (D + 1):tkb * VSTR + (i + 1) * (D + 1)
                    ]

                pending = []

                def flush(depth=0, last=False):
                    while len(pending) > depth:
                        P, items = pending.pop(0)
                        for idx, (tkb, oc0, pc0, w) in enumerate(items):
                            nc.tensor.matmul(
                                y_ps[:, oc0:oc0 + w],
                                vslice(tkb),
                                P[:, pc0:pc0 + w],
                                start=first_pv[0],
                                stop=(last and not pending and idx == len(items) - 1),
                                skip_group_check=True,
                            )
                            first_pv[0] = False

                def group(items):
                    total = items[-1][2] + items[-1][3]
                    s_g = ps.tile([128, 1024], F32, name="s_g", tag="mm", bufs=3)
                    P_g = pw.tile([128, 1024], F16, name="P_g", tag="P_t", bufs=8)
                    for tkb, oc0, pc0, w, _ in items:
                        nc.tensor.matmul(
                            s_g[:, pc0:pc0 + w],
                            kT[m][p0:p0 + 64, tkb * 128:(tkb + 1) * 128],
                            qT[m][p0:p0 + 64, c * 512 + oc0:(c + 1) * 512][:, 0:w],
                            start=True,
                            stop=True,
                        )
                    nc.scalar.activation(
                        P_g[:, 0:total], s_g[:, 0:total], Exp, scale=0.125)
                    for tkb, oc0, pc0, w, straddler in items:
                        if straddler:
                            nc.gpsimd.affine_select(
                                out=P_g[:, pc0:pc0 + w],
                                in_=P_g[:, pc0:pc0 + w],
                                compare_op=is_ge,
                                fill=0.0,
                                base=0,
                                pattern=[[1, w]],
                                channel_multiplier=-1,
                            )
                    flush(depth=3)
                    pending.append((P_g, [it[:4] for it in items]))

                group([(4 * c, 0, 0, 512, True),
                       (4 * c + 1, 128, 512, 384, True)])
                group([(4 * c + 2, 256, 0, 256, True),
                       (4 * c + 3, 384, 256, 128, True)])
                for pair in range(2 * c):
                    group([(2 * pair, 0, 0, 512, False),
                           (2 * pair + 1, 0, 512, 512, False)])
                flush(last=True)
                return [(y_ps, m, p0, c, 0)]

            def attention(c, j):
                """Head pair (2j, 2j+1): row-halves 0-63 / 64-127 of tile j.
                Scores emitted as adjacent K=64 matmuls (concurrent on HW);
                one 1024-col PSUM + one exp covers both heads. Returns pends
                for deferred normalization."""
                iA, iB = 2 * j, 2 * j + 1
                # ONE psum tile for the pair: head A accumulates in bank 0
                # (cols 0-511), head B in bank 1 -- a single open PSUM
                # accumulation stream on PE (two interleaved open groups
                # break walrus codegen / hang the device).
                y_ps2 = ps.tile([D + 1, 1024], F32, name="y_ps2", tag="psy", bufs=1)
                # PSUM start=True marks pending-zero per 2KB ZERO REGION (one
                # bank): each head's 512-col bank needs its own start on its
                # first PV (and stop on its last, for the sim).
                npvh = 4 + 4 * c            # PV matmuls per head
                cnt = {iA: 0, iB: 0}

                def vslice(tkb, i):
                    return v_ext[
                        :, tkb * VSTR + i * (D + 1):tkb * VSTR + (i + 1) * (D + 1)
                    ]

                pending = []  # [(P tile, [(tkb, head, out_col0, p_col0, w), ...])]

                def flush(depth=0):
                    while len(pending) > depth:
                        P, items = pending.pop(0)
                        for tkb, i, oc0, pc0, w in items:
                            off = 0 if i == iA else 512
                            n = cnt[i]
                            nc.tensor.matmul(
                                y_ps2[:, off + oc0:off + oc0 + w],
                                vslice(tkb, i),
                                P[:, pc0:pc0 + w],
                                start=(n == 0),
                                stop=(n == npvh - 1),
                                skip_group_check=True,
                            )
                            cnt[i] = n + 1

                def group(items, sels, exp_w2=None):
                    """items: (kb, head, q_off, width, p_col0) -- every
                    [pc0, pc0+w) must stay inside one 512-col PSUM bank, and
                    ADJACENT row-alternating matmuls must write DIFFERENT
                    banks (concurrent drains into one bank break the device).
                    sels: (j0, w, stride) -> affine_select over the cols
                    {b*stride + j0 .. +w} for b in 0..1 (two blocks per pair,
                    same local causal pattern). exp_w2: if set, exp covers
                    cols {0..exp_w2} u {512..512+exp_w2} instead of a
                    contiguous prefix."""
                    total = items[-1][4] + items[-1][3]
                    s_g = ps.tile([128, 1024], F32, name="s_g", tag="mm", bufs=3)
                    P_g = pw.tile([128, 1024], F16, name="P_g", tag="P_t", bufs=8)
                    for kb, i, qoff, w, pc0 in items:
                        p0 = 64 * (i % 2)
                        nc.tensor.matmul(
                            s_g[:, pc0:pc0 + w],
                            kT[j][p0:p0 + 64, kb * 128:(kb + 1) * 128],
                            qT[j][p0:p0 + 64, c * 512 + qoff:c * 512 + qoff + w],
                            start=True,
                            stop=True,
                        )
                    if exp_w2 is not None:
                        nc.scalar.activation(
                            P_g[:].rearrange("p (b cc) -> p b cc", cc=512)[:, 0:2, 0:exp_w2],
                            s_g[:].rearrange("p (b cc) -> p b cc", cc=512)[:, 0:2, 0:exp_w2],
                            Exp, scale=0.125)
                    else:
                        nc.scalar.activation(
                            P_g[:, 0:total], s_g[:, 0:total], Exp, scale=0.125)
                    for j0, w, stride in sels:
                        # keep where (local col within each w-span) >= partition
                        for bb in range(2 if stride else 1):
                            view = P_g[:, bb * stride + j0:bb * stride + j0 + w]
                            nc.gpsimd.affine_select(
                                out=view,
                                in_=view,
                                compare_op=is_ge,
                                fill=0.0,
                                base=0,
                                pattern=[[1, w]],
                                channel_multiplier=-1,
                            )
                    flush(depth=3)
                    pending.append(
                        (P_g, [(kb, i, qoff, pc0, w) for kb, i, qoff, w, pc0 in items]))

                kb0 = 4 * c
                if GMODE == "pair_g13":
                    group([(kb0, iA, 0, 512, 0), (kb0, iB, 0, 512, 512)],
                          sels=[(0, 512, 512)])
                    group([(kb0 + 2, iA, 256, 256, 0), (kb0 + 2, iB, 256, 256, 256)],
                          sels=[(0, 256, 256)])
                    for i in (iA, iB):
                        group([(kb0 + 1, i, 128, 384, 0), (kb0 + 3, i, 384, 128, 384)],
                              sels=[(0, 384, 0), (384, 128, 0)])
                    for kb in range(4 * c):
                        group([(kb, iA, 0, 512, 0), (kb, iB, 0, 512, 512)],
                              sels=[])
                elif GMODE in ("pair_diag", "pair_full"):
                    if GMODE == "pair_diag":
                        group([(kb0, iA, 0, 512, 0), (kb0, iB, 0, 512, 512)],
                              sels=[(0, 512, 512)])
                        group([(kb0 + 1, iA, 128, 384, 0), (kb0 + 1, iB, 128, 384, 512),
                               (kb0 + 3, iA, 384, 128, 384), (kb0 + 3, iB, 384, 128, 896)],
                              sels=[(0, 384, 512), (384, 128, 512)])
                        group([(kb0 + 2, iA, 256, 256, 0), (kb0 + 2, iB, 256, 256, 256)],
                              sels=[(0, 256, 256)])
                        for i in (iA, iB):
                            for pr in range(2 * c):
                                group([(2 * pr, i, 0, 512, 0),
                                       (2 * pr + 1, i, 0, 512, 512)], sels=[])
                    else:
                        for i in (iA, iB):
                            group([(kb0, i, 0, 512, 0), (kb0 + 1, i, 128, 384, 512)],
                                  sels=[(0, 512, 0), (512, 384, 0)])
                            group([(kb0 + 2, i, 256, 256, 0), (kb0 + 3, i, 384, 128, 256)],
                                  sels=[(0, 256, 0), (256, 128, 0)])
                        for kb in range(4 * c):
                            group([(kb, iA, 0, 512, 0), (kb, iB, 0, 512, 512)],
                                  sels=[])
                elif GMODE == "pair":
                    # diagonal straddlers first (their Pool affine_selects get
                    # a full pipeline depth of slack before their PV is due);
                    # A blocks fill bank 0 (cols 0-511), B blocks bank 1 --
                    # adjacent alternating-row matmuls always hit opposite
                    # banks.
                    group([(kb0, iA, 0, 512, 0), (kb0, iB, 0, 512, 512)],
                          sels=[(0, 512, 512)])
                    group([(kb0 + 1, iA, 128, 384, 0), (kb0 + 1, iB, 128, 384, 512),
                           (kb0 + 3, iA, 384, 128, 384), (kb0 + 3, iB, 384, 128, 896)],
                          sels=[(0, 384, 512), (384, 128, 512)])
                    group([(kb0 + 2, iA, 256, 256, 0), (kb0 + 2, iB, 256, 256, 512)],
                          sels=[(0, 256, 512)], exp_w2=256)
                    # full blocks: one kb per group, both heads
                    for kb in range(4 * c):
                        group([(kb, iA, 0, 512, 0), (kb, iB, 0, 512, 512)],
                              sels=[])
                else:
                    # per-head v1-style packing into the shared pair psum
                    for i in (iA, iB):
                        group([(kb0, i, 0, 512, 0), (kb0 + 1, i, 128, 384, 512)],
                              sels=[(0, 512, 0), (512, 384, 0)])
                        group([(kb0 + 2, i, 256, 256, 0), (kb0 + 3, i, 384, 128, 256)],
                              sels=[(0, 256, 0), (256, 128, 0)])
                        for pr in range(2 * c):
                            group([(2 * pr, i, 0, 512, 0),
                                   (2 * pr + 1, i, 0, 512, 512)], sels=[])
                flush()
                return [(y_ps2, j, 0, c, 0), (y_ps2, j, 64, c, 512)]

            def normalize(pend):
                y_ps, m, p0, c, col0 = (pend if len(pend) == 5
                                        else (*pend, 0))
                r_row = pw.tile([1, 512], F16, name="r_row", tag="r_row", bufs=2)
                with nc.allow_low_precision(reason="fp16 matches PE fp22 input precision"):
                    nc.vector.reciprocal(
                        r_row[0:1, :], y_ps[D:D + 1, col0:col0 + 512])
                R_sb = pw.tile([64, 512], F16, name="R_sb", tag="R_sb", bufs=2)
                nc.gpsimd.partition_broadcast(R_sb[:], r_row[0:1, :])
                with nc.allow_low_precision(reason="fp16 matches PE fp22 input precision"):
                    nc.vector.tensor_mul(
                        yT[m][p0:p0 + 64, c * 512:(c + 1) * 512],
                        y_ps[0:D, col0:col0 + 512],
                        R_sb[:],
                    )

            def proj(mt, oq):
                o_t = pw.tile([128, C], F16, name="o_t", tag="o_t", bufs=2)
                for nn in range(2):
                    pj_ps = ps.tile([128, 512], F32, name="pj_ps", tag="mm", bufs=3)
                    for kk in range(4):
                        nc.tensor.matmul(
                            pj_ps[:],
                            yT[kk][:, mt * 128:(mt + 1) * 128],
                            w_pj_sb[:, kk * 1024 + nn * 512:kk * 1024 + (nn + 1) * 512],
                            start=(kk == 0),
                            stop=(kk == 3),
                        )
                    with nc.allow_low_precision(reason="fp16 output transport"):
                        nc.vector.tensor_copy(o_t[:, nn * 512:(nn + 1) * 512], pj_ps[:])
                oq.dma_start(out=out[mt * 128:(mt + 1) * 128, :], in_=o_t[:])

            # ---- emission schedule ----
            # phase 1 chunk 0 up front; chunk c+1 and proj of chunk c-1 drip
            # through attention chunk c.
            steps0 = list(phase1_chunk_steps(0, nc.sync))
            steps0[0]()          # x chunk-0 load, right after w_qk slabs
            for s in steps0[1:]:
                s()

            nc.scalar.dma_start(
                out=w_pj_sb[:, 0:2048], in_=w_pj[:, 0:2048])
            nc.sync.dma_start(
                out=w_pj_sb[:, 2048:4096], in_=w_pj[:, 2048:4096])

            fillers = []         # queue of deferred PE-work closures
            oqrr = [0]

            def next_oq():
                oqrr[0] = (oqrr[0] + 1) % 2
                return qs[oqrr[0]]

            for c in range(NCH):
                # stage fillers for this attention chunk
                if c + 1 < NCH:
                    fillers.extend(phase1_chunk_steps(c + 1, nc.sync))
                if c == NCH - 1:
                    # last chunk: all remaining proj work becomes filler
                    for mt in range(4 * (NCH - 1)):
                        fillers.append(lambda mt=mt: proj(mt, next_oq()))
                nsteps = len(fillers)
                # spread fillers evenly across the 4 pairs of this chunk
                for jj in range(4):
                    take = nsteps * (jj + 1) // 4 - nsteps * jj // 4
                    if PAIRED:
                        for p in attention(c, jj):
                            normalize(p)
                    else:
                        for i in (2 * jj, 2 * jj + 1):
                            for p in attention_v1(c, i):
                                normalize(p)
                    for _ in range(take):
                        if fillers:
                            fillers.pop(0)()
            for f in fillers:
                f()
            for mt in range(12, 16):
                proj(mt, next_oq())

    nc.compile()
    return nc


_NC = None


def _get_nc():
    global _NC
    if _NC is None:
        _NC = build_nc()
    return _NC


def make_in_maps(x, w_attn, b_attn, w_proj):
    x = np.asarray(x, dtype=np.float32)
    w_attn = np.asarray(w_attn, dtype=np.float32)
    b_attn = np.asarray(b_attn, dtype=np.float32)
    w_proj = np.asarray(w_proj, dtype=np.float32)
    in_maps = []
    for core in range(8):
        b, g = divmod(core, 2)
        s = g * CQ
        # xT chunk-major: [128, ct*4096 + kk*512 + t]
        xt = np.ascontiguousarray(
            x[b].reshape(NCH, TCH, 8, 128).transpose(3, 0, 2, 1)
        ).reshape(128, NCH * 8 * TCH)
        wqk = np.concatenate(
            [w_attn[:, s:s + CQ], w_attn[:, C + s:C + s + CQ]], axis=1
        )  # [1024, 1024]
        wv = w_attn[:, 2 * C + s:2 * C + s + CQ]  # [1024, 512]
        in_maps.append({
            "xT": xt.astype(np.float16),
            "w_qk": np.ascontiguousarray(
                wqk.reshape(8, 128, 1024).transpose(1, 0, 2).reshape(128, 8192)
            ).astype(np.float16),
            "w_v": np.ascontiguousarray(
                wv.reshape(8, 128, 512).transpose(1, 0, 2).reshape(128, 4096)
            ).astype(np.float16),
            "b_qk": np.ascontiguousarray(
                np.concatenate([b_attn[s:s + CQ], b_attn[C + s:C + s + CQ]])
                .reshape(8, 128).T
            ).astype(np.float32),
            "b_vb": np.broadcast_to(
                b_attn[2 * C + s:2 * C + s + CQ], (128, CQ)
            ).astype(np.float16),
            "w_pj": np.ascontiguousarray(
                w_proj[s:s + CQ, :].reshape(4, 128, 1024)
                .transpose(1, 0, 2).reshape(128, 4096)
            ).astype(np.float16),
        })
    return in_maps


def kernel(x, w_attn, b_attn, w_proj, b_proj):
    nc = _get_nc()
    in_maps = make_in_maps(x, w_attn, b_attn, w_proj)
    res = run_bass_kernel_spmd(nc, in_maps, list(range(8)))
    b_proj = np.asarray(b_proj, dtype=np.float32)
    out = np.empty((B, T, C), dtype=np.float32)
    for b in range(B):
        out[b] = (res.results[2 * b]["out"].astype(np.float32)
                  + res.results[2 * b + 1]["out"].astype(np.float32) + b_proj)
    return out


# revision 37
# speedup vs baseline: 1.1787x; 1.1787x over previous
"""Causal self-attention (B=4, T=2048, C=1024, H=16) on 8 NeuronCores.

Sharding: data-parallel over batch (4) x tensor-parallel over heads (2 groups
of 8 heads) = 8 cores. Each core computes QKV for its 8 heads, causal
attention, and a partial output projection (row-parallel). Host sums the two
partial projections per batch and adds b_proj.

All matmul operands fp16 (PE multiplies at ~fp22 internally; fp16's 11-bit
mantissa keeps rel-err ~5e-4, well under the 2e-2 gate), fp32 PSUM accum.

HW facts this kernel is built around (all measured on the device):
  - every self-loading matmul pays ~110-150ns of serial LDWEIGHTS+dispatch on
    top of N/2.4GHz streaming (ldw-opt is disabled in walrus), and K=64
    matmuls pay DOUBLE weight-load (no FWL below 128 rows);
  - BUT two K=64 matmuls on alternating row-halves execute CONCURRENTLY
    (2 cols/cycle, ~382ns/pair@N=512 vs ~980ns unpaired) -- so score matmuls
    for the head pair (2j, 2j+1), which live on row-halves 0-63/64-127 of
    the same kT/qT tile, are emitted adjacently and run ~2.5x faster;
  - adjacent row-alternating matmuls MUST write different PSUM banks
    (concurrent drains into one bank wedge the device), and a PSUM
    accumulation needs start=True per 2KB zero-region (bank) it touches;
  - ACT exp costs ~340-390ns fixed per instruction + 0.83ns/col, so exp
    batches 1024 cols (both heads of a pair share one s_g tile + one exp);
  - the PE queue is in-order: filler work (QKV of the next T-chunk, the
    projection of finished chunks) is interleaved BETWEEN score groups, and
    the PV pipeline (depth 4) runs as one global queue across pairs/chunks
    so the exp->PV latency is never exposed at pair boundaries.

Other structure:
  - x is pre-transposed on the HOST and loaded per 512-row chunk as one
    contiguous DMA (no device DMA-transpose); weights load as slabs spread
    over the SP/ACT queues so the first matmul starts ~2us in; out-DMAs
    round-robin over SP/Pool (never ACT -- exp must not queue behind them).
  - v carries a ones-column so the PV matmul emits softmax denominators
    (row 64 of the pair's [65,1024] PSUM tile); normalization = one DVE
    reciprocal + one Pool partition-broadcast + two DVE multiplies per pair,
    emitted one pair behind the PV stream (PSUM: 2x 1024-col "mm" bufs +
    2x [65,1024] "psy" bufs = 8 banks exactly).
  - causal masking: diagonal blocks restrict score columns; the remaining
    straddle is zero-filled by Pool affine_selects right after the exp.
"""

from contextlib import nullcontext

import numpy as np

import concourse.bass as bass
import concourse.mybir as mybir
from concourse import bacc
from concourse.tile import TileContext
from concourse.bass_utils import run_bass_kernel_spmd

B, T, C, H, D = 4, 2048, 1024, 16, 64
CQ = 512          # q (or k or v) channels per core = 8 heads * 64
HPC = 8           # heads per core
F32 = mybir.dt.float32
F16 = mybir.dt.float16
Exp = mybir.ActivationFunctionType.Exp
is_ge = mybir.AluOpType.is_ge

TCH = 512         # phase-1 T-chunk
NCH = T // TCH    # 4 chunks
VSTR = HPC * (D + 1)   # 520: v_ext per-T-block stride (8 heads x 65)


def build_nc(loop_n=1):
    """loop_n > 1 wraps the whole kernel in a device-side repeat loop
    (benchmarking only -- output is identical every iteration)."""
    nc = bacc.Bacc("TRN2", target_bir_lowering=False, debug=False, num_devices=8)

    # host-pretransposed x, chunk-major: xT[p, ct*4096 + kk*512 + t]
    xT = nc.dram_tensor("xT", [128, NCH * 8 * TCH], F16, kind="ExternalInput")
    # w_qk packed [p, kk*1024 + n] = w_qk[kk*128+p, n]
    w_qk = nc.dram_tensor("w_qk", [128, 8 * 1024], F16, kind="ExternalInput")
    w_v = nc.dram_tensor("w_v", [128, 8 * 512], F16, kind="ExternalInput")
    b_qk = nc.dram_tensor("b_qk", [128, 8], F32, kind="ExternalInput")
    b_vb = nc.dram_tensor("b_vb", [128, CQ], F16, kind="ExternalInput")
    w_pj = nc.dram_tensor("w_pj", [128, 4 * 1024], F16, kind="ExternalInput")
    out = nc.dram_tensor("out", [T, C], F16, kind="ExternalOutput")

    with TileContext(nc) as tc:
        with (
            tc.tile_pool(name="const", bufs=1) as pc,
            tc.tile_pool(name="persist", bufs=1) as pp,
            tc.tile_pool(name="work", bufs=2) as pw,
            tc.tile_pool(name="psum", bufs=2, space="PSUM") as ps,
            tc.For_i(0, loop_n, 1) if loop_n > 1 else nullcontext(),
        ):
            # ---- persistent activations ----
            qT = [pp.tile([128, T], F16, name=f"qT{m}") for m in range(4)]
            kT = [pp.tile([128, T], F16, name=f"kT{m}") for m in range(4)]
            yT = [pp.tile([128, T], F16, name=f"yT{m}") for m in range(4)]
            v_ext = pp.tile([128, (T // 128) * VSTR], F16, name="v_ext")

            w_qk_sb = pc.tile([128, 8 * 1024], F16, name="w_qk_sb")
            b_qk2 = pc.tile([128, 8], F32, name="b_qk2")
            b_vb_sb = pc.tile([128, CQ], F16, name="b_vb_sb")
            w_v_sb = pc.tile([128, 8 * 512], F16, name="w_v_sb")
            w_pj_sb = pc.tile([128, 4 * 1024], F16, name="w_pj_sb")

            # DMA queue round-robin (keeps any single DGE queue from
            # serializing the kernel)
            qs = [nc.sync, nc.scalar]

            # ---- startup DMAs: first matmul needs w_qk slab 0 + x chunk 0
            xc0 = pw.tile([128, 8 * TCH], F16, name="xT_c", tag="xT_c", bufs=2)
            nc.sync.dma_start(out=xc0[:, 0:2048], in_=xT[:, 0:2048])
            nc.scalar.dma_start(out=xc0[:, 2048:4096], in_=xT[:, 2048:4096])
            for kk in range(8):
                qs[kk % 2].dma_start(
                    out=w_qk_sb[:, kk * 1024:(kk + 1) * 1024],
                    in_=w_qk[:, kk * 1024:(kk + 1) * 1024],
                )
            nc.sync.dma_start(
                out=w_v_sb[:], in_=w_v[:])
            nc.scalar.dma_start(out=b_qk2[:], in_=b_qk[:])
            nc.scalar.dma_start(out=b_vb_sb[:], in_=b_vb[:])

            # ones column of v_ext makes PV emit softmax denominators
            v_ones = v_ext[:].rearrange(
                "p (t i d) -> p t i d", i=HPC, d=D + 1
            )[:, :, :, D:D + 1]
            nc.gpsimd.memset(v_ones, 1.0)

            def phase1_chunk_steps(ct, xq, pre=None):
                """Yield emission closures for one phase-1 chunk."""
                T0 = ct * TCH
                xT_c = pre if pre is not None else pw.tile(
                    [128, 8 * TCH], F16, name="xT_c", tag="xT_c", bufs=2)

                def xload():
                    if pre is None:
                        xq.dma_start(
                            out=xT_c[:],
                            in_=xT[:, ct * 8 * TCH:(ct + 1) * 8 * TCH],
                        )

                def qk(m0):
                    # q,k: out^T layout [ch, T-chunk]; bias fused into copy
                    for m in range(m0, m0 + 2):
                        qk_ps = ps.tile([128, TCH], F32, name="qk_ps", tag="mm", bufs=2)
                        for kk in range(8):
                            nc.tensor.matmul(
                                qk_ps[:],
                                w_qk_sb[:, kk * 1024 + m * 128:kk * 1024 + (m + 1) * 128],
                                xT_c[:, kk * TCH:(kk + 1) * TCH],
                                start=(kk == 0),
                                stop=(kk == 7),
                            )
                        dst = qT[m] if m < 4 else kT[m - 4]
                        nc.vector.tensor_scalar_add(
                            dst[:, T0:T0 + TCH], qk_ps[:], b_qk2[:, m:m + 1]
                        )

                def vpart(tt):
                    # v: natural layout [T-block, ch], interleaved into v_ext
                    v_ps = ps.tile([128, CQ], F32, name="v_ps", tag="mm", bufs=2)
                    for kk in range(8):
                        nc.tensor.matmul(
                            v_ps[:],
                            xT_c[:, kk * TCH + tt * 128:kk * TCH + (tt + 1) * 128],
                            w_v_sb[:, kk * 512:(kk + 1) * 512],
                            start=(kk == 0),
                            stop=(kk == 7),
                        )
                    tb = ct * (TCH // 128) + tt
                    dst = v_ext[:, tb * VSTR:(tb + 1) * VSTR].rearrange(
                        "p (i d) -> p i d", d=D + 1
                    )[:, :, 0:D]
                    src = v_ps[:].rearrange("p (i d) -> p i d", d=D)
                    bsrc = b_vb_sb[:].rearrange("p (i d) -> p i d", d=D)
                    nc.vector.tensor_add(dst, src, bsrc)

                yield xload
                yield lambda: qk(0)
                yield lambda: qk(2)
                yield lambda: qk(4)
                yield lambda: qk(6)
                for t0 in range(0, TCH // 128):
                    yield lambda t0=t0: vpart(t0)

            pv_pend = []   # global PV pipeline: (pair, P_g, [(y, col0, w, ...)])

            def pv_emit1():
                _, P, items = pv_pend.pop(0)
                for y, col0, w, vsl, pc0, st, sp in items:
                    nc.tensor.matmul(
                        y[:, col0:col0 + w],
                        vsl,
                        P[:, pc0:pc0 + w],
                        start=st,
                        stop=sp,
                        skip_group_check=True,
                    )

            def pv_flush(depth=0):
                while len(pv_pend) > depth:
                    pv_emit1()

            def pv_clear(pair):
                # a pair's normalize may only be emitted once all its PVs are
                # (c=0 pairs have just 3 groups -- the pipeline can still
                # hold their tail)
                while pv_pend and pv_pend[0][0] == pair:
                    pv_emit1()

            def attention(c, j, on_group=None):
                """Head pair (2j, 2j+1) = row-halves 0-63 / 64-127 of tile j.
                Scores per group: adjacent K=64 matmuls on alternating
                row-halves (concurrent on HW) writing OPPOSITE PSUM banks of
                one [128,1024] tile (adjacent row-alternating matmuls into
                one bank wedge the device). One exp covers both heads.
                PV accumulates into a shared [65,1024] pair tile: head A
                bank 0, head B bank 1, per-head start/stop (PSUM start
                marks pending-zero per 2KB zero region = one bank)."""
                iA, iB = 2 * j, 2 * j + 1
                y_ps2 = ps.tile([D + 1, 1024], F32, name="y_ps2", tag="psy", bufs=2)
                npvh = 4 + 4 * c            # PV matmuls per head
                cnt = {iA: 0, iB: 0}

                def vslice(tkb, i):
                    return v_ext[
                        :, tkb * VSTR + i * (D + 1):tkb * VSTR + (i + 1) * (D + 1)
                    ]

                def group(items, sels=(), exp_w2=None):
                    """items: (kb, head, q_off, width, p_col0); every
                    [pc0,pc0+w) inside one bank; adjacent items alternate
                    heads AND banks. sels: (j0, w, stride) -> affine_select
                    over cols {b*stride + j0 ..+w}, b in 0..1 (stride=0: one
                    span). exp_w2: exp covers {0..w2} u {512..512+w2}."""
                    total = items[-1][4] + items[-1][3]
                    s_g = ps.tile([128, 1024], F32, name="s_g", tag="mm", bufs=2)
                    P_g = pw.tile([128, 1024], F16, name="P_g", tag="P_t", bufs=8)
                    for kb, i, qoff, w, pc0 in items:
                        p0 = 64 * (i % 2)
                        nc.tensor.matmul(
                            s_g[:, pc0:pc0 + w],
                            kT[j][p0:p0 + 64, kb * 128:(kb + 1) * 128],
                            qT[j][p0:p0 + 64, c * 512 + qoff:c * 512 + qoff + w],
                            start=True,
                            stop=True,
                        )
                    if exp_w2 is not None:
                        nc.scalar.activation(
                            P_g[:].rearrange("p (b cc) -> p b cc", cc=512)[:, 0:2, 0:exp_w2],
                            s_g[:].rearrange("p (b cc) -> p b cc", cc=512)[:, 0:2, 0:exp_w2],
                            Exp, scale=0.125)
                    else:
                        nc.scalar.activation(
                            P_g[:, 0:total], s_g[:, 0:total], Exp, scale=0.125)
                    for j0, w, stride in sels:
                        # keep where local col within each span >= partition
                        for bb in range(2 if stride else 1):
                            view = P_g[:, bb * stride + j0:bb * stride + j0 + w]
                            nc.gpsimd.affine_select(
                                out=view,
                                in_=view,
                                compare_op=is_ge,
                                fill=0.0,
                                base=0,
                                pattern=[[1, w]],
                                channel_multiplier=-1,
                            )
                    pv_items = []
                    for kb, i, qoff, w, pc0 in items:
                        n = cnt[i]
                        cnt[i] = n + 1
                        pv_items.append((
                            y_ps2, (0 if i == iA else 512) + qoff, w,
                            vslice(kb, i), pc0, n == 0, n == npvh - 1))
                    pv_flush(depth=4)
                    pv_pend.append(((c, j), P_g, pv_items))
                    if on_group is not None:
                        on_group()

                kb0 = 4 * c
                # diagonal straddlers first (their Pool affine_selects get a
                # full pipeline depth of slack before their PV is due);
                # A blocks in bank 0 (cols 0-511), B in bank 1
                group([(kb0, iA, 0, 512, 0), (kb0, iB, 0, 512, 512)],
                      sels=[(0, 512, 512)])
                group([(kb0 + 1, iA, 128, 384, 0), (kb0 + 1, iB, 128, 384, 512),
                       (kb0 + 3, iA, 384, 128, 384), (kb0 + 3, iB, 384, 128, 896)],
                      sels=[(0, 384, 512), (384, 128, 512)])
                group([(kb0 + 2, iA, 256, 256, 0), (kb0 + 2, iB, 256, 256, 512)],
                      sels=[(0, 256, 512)], exp_w2=256)
                # full (below-diagonal) blocks: one kb per group, both heads
                for kb in range(4 * c):
                    group([(kb, iA, 0, 512, 0), (kb, iB, 0, 512, 512)])
                return [(y_ps2, j, c)]

            def normalize_pair(y_ps, m, c):
                # both heads' denominators live in partition row 64 (A: cols
                # 0-511, B: 512-1023) -> one reciprocal + one broadcast
                r_row = pw.tile([1, 1024], F16, name="r_row", tag="r_row", bufs=2)
                with nc.allow_low_precision(reason="fp16 matches PE fp22 input precision"):
                    nc.vector.reciprocal(r_row[0:1, :], y_ps[D:D + 1, :])
                R_sb = pw.tile([64, 1024], F16, name="R_sb", tag="R_sb", bufs=2)
                nc.gpsimd.partition_broadcast(R_sb[:], r_row[0:1, :])
                with nc.allow_low_precision(reason="fp16 matches PE fp22 input precision"):
                    nc.vector.tensor_mul(
                        yT[m][0:64, c * 512:(c + 1) * 512],
                        y_ps[0:D, 0:512],
                        R_sb[:, 0:512],
                    )
                    nc.vector.tensor_mul(
                        yT[m][64:128, c * 512:(c + 1) * 512],
                        y_ps[0:D, 512:1024],
                        R_sb[:, 512:1024],
                    )

            proj_ot = [None]

            def proj_nn(mt, nn, oq):
                if nn == 0:
                    proj_ot[0] = pw.tile([128, C], F16, name="o_t", tag="o_t", bufs=2)
                o_t = proj_ot[0]
                pj_ps = ps.tile([128, 512], F32, name="pj_ps", tag="mm", bufs=2)
                for kk in range(4):
                    nc.tensor.matmul(
                        pj_ps[:],
                        yT[kk][:, mt * 128:(mt + 1) * 128],
                        w_pj_sb[:, kk * 1024 + nn * 512:kk * 1024 + (nn + 1) * 512],
                        start=(kk == 0),
                        stop=(kk == 3),
                    )
                with nc.allow_low_precision(reason="fp16 output transport"):
                    nc.vector.tensor_copy(o_t[:, nn * 512:(nn + 1) * 512], pj_ps[:])
                if nn == 1:
                    oq.dma_start(out=out[mt * 128:(mt + 1) * 128, :], in_=o_t[:])

            # ---- emission schedule ----
            # phase 1 chunk 0 up front; chunk c+1 and proj of chunk c-1 drip
            # through attention chunk c.
            steps0 = list(phase1_chunk_steps(0, nc.sync, pre=xc0))
            steps0[0]()          # x chunk-0 load, right after w_qk slabs
            for s in steps0[1:]:
                s()

            nc.scalar.dma_start(
                out=w_pj_sb[:, 0:2048], in_=w_pj[:, 0:2048])
            nc.sync.dma_start(
                out=w_pj_sb[:, 2048:4096], in_=w_pj[:, 2048:4096])

            fillers = []         # queue of deferred PE-work closures
            pends = []           # previous pair's normalizations (lag-1)
            # out-DMAs stay OFF the scalar queue: a DMA there would stall
            # exp instructions (ACT is the co-critical engine in attention)
            oqueues = [nc.sync, nc.gpsimd]
            oqrr = [0]

            def next_oq():
                oqrr[0] = (oqrr[0] + 1) % 2
                return oqueues[oqrr[0]]

            for c in range(NCH):
                # stage fillers for this attention chunk
                if c + 1 < NCH:
                    fillers.extend(phase1_chunk_steps(c + 1, nc.sync))
                if c == NCH - 1:
                    # last chunk: all remaining proj work becomes filler
                    for mt in range(4 * (NCH - 1)):
                        for nn in range(2):
                            fillers.append(
                                lambda mt=mt, nn=nn: proj_nn(mt, nn, next_oq()))
                # pace fillers evenly over this chunk's score groups; the PE
                # queue is in-order, so filler must be emitted BETWEEN groups
                # (a filler blob behind a stalled scores matmul is useless)
                ngroups = 4 * (3 + 4 * c)
                pace = len(fillers) / ngroups
                acc = [0.0]

                def on_group():
                    acc[0] += pace
                    while acc[0] >= 1.0 and fillers:
                        fillers.pop(0)()
                        acc[0] -= 1.0

                for jj in range(4):
                    nxt = attention(c, jj, on_group)
                    while pends:
                        y_ps, m, cc = pends.pop(0)
                        pv_clear((cc, m))
                        normalize_pair(y_ps, m, cc)
                    pends = nxt
                # chunk boundary: drain the PV pipeline and normalize the
                # last pair BEFORE the next chunk's fillers may read its yT
                pv_flush()
                while pends:
                    normalize_pair(*pends.pop(0))
            pv_flush()
            while pends:
                normalize_pair(*pends.pop(0))
            for f in fillers:
                f()
            for mt in range(12, 16):
                for nn in range(2):
                    proj_nn(mt, nn, next_oq())

    nc.compile()
    return nc


_NC = None


def _get_nc():
    global _NC
    if _NC is None:
        _NC = build_nc()
    return _NC


def make_in_maps(x, w_attn, b_attn, w_proj):
    x = np.asarray(x, dtype=np.float32)
    w_attn = np.asarray(w_attn, dtype=np.float32)
    b_attn = np.asarray(b_attn, dtype=np.float32)
    w_proj = np.asarray(w_proj, dtype=np.float32)
    in_maps = []
    for core in range(8):
        b, g = divmod(core, 2)
        s = g * CQ
        # xT chunk-major: [128, ct*4096 + kk*512 + t]
        xt = np.ascontiguousarray(
            x[b].reshape(NCH, TCH, 8, 128).transpose(3, 0, 2, 1)
        ).reshape(128, NCH * 8 * TCH)
        wqk = np.concatenate(
            [w_attn[:, s:s + CQ], w_attn[:, C + s:C + s + CQ]], axis=1
        )  # [1024, 1024]
        wv = w_attn[:, 2 * C + s:2 * C + s + CQ]  # [1024, 512]
        in_maps.append({
            "xT": xt.astype(np.float16),
            "w_qk": np.ascontiguousarray(
                wqk.reshape(8, 128, 1024).transpose(1, 0, 2).reshape(128, 8192)
            ).astype(np.float16),
            "w_v": np.ascontiguousarray(
                wv.reshape(8, 128, 512).transpose(1, 0, 2).reshape(128, 4096)
            ).astype(np.float16),
            "b_qk": np.ascontiguousarray(
                np.concatenate([b_attn[s:s + CQ], b_attn[C + s:C + s + CQ]])
                .reshape(8, 128).T
            ).astype(np.float32),
            "b_vb": np.broadcast_to(
                b_attn[2 * C + s:2 * C + s + CQ], (128, CQ)
            ).astype(np.float16),
            "w_pj": np.ascontiguousarray(
                w_proj[s:s + CQ, :].reshape(4, 128, 1024)
                .transpose(1, 0, 2).reshape(128, 4096)
            ).astype(np.float16),
        })
    return in_maps


def kernel(x, w_attn, b_attn, w_proj, b_proj):
    nc = _get_nc()
    in_maps = make_in_maps(x, w_attn, b_attn, w_proj)
    res = run_bass_kernel_spmd(nc, in_maps, list(range(8)))
    b_proj = np.asarray(b_proj, dtype=np.float32)
    out = np.empty((B, T, C), dtype=np.float32)
    for b in range(B):
        out[b] = (res.results[2 * b]["out"].astype(np.float32)
                  + res.results[2 * b + 1]["out"].astype(np.float32) + b_proj)
    return out
